# revision 39
# baseline (speedup 1.0000x reference)
"""Trainium2 Bass kernel: 3D-RoPE multi-head attention (B=4,N=2048,DIM=1536,H=16,DH=96).

Sharding: 8 cores = (batch b = c//2) x (head half hh = c%2, 8 heads each).
Each core, for its batch and its 8 heads:
  - projects Q,K (transposed layout [DH, N], RoPE applied) and V (natural)
  - attention over all 2048 tokens (softmax via ones-column denominator)
  - partial output projection with its 8 heads' rows of Wout
Host sums the two partial outputs per batch and adds bout.

All tensors live in SBUF (no DRAM spills). Attention for head h-2 is
interleaved with the projection of head h so the scalar-engine exp stream
overlaps PE projection work. Attention processes query-tile PAIRS per key
chunk (each K/V stationary load feeds two matmuls; exp on [128,1024]
tiles). The tail runs the last two heads' attention with 3-deep score
buffering (scalar runahead), then a dense output projection that
contracts at full PE width over DMA-repacked 128-row attention outputs.
"""

import sys

if "/opt/trn_rl_repo" not in sys.path:
    sys.path.insert(0, "/opt/trn_rl_repo")

import numpy as np
import ml_dtypes

import concourse.bass as bass
import concourse.mybir as mybir
import concourse.tile as tile
from concourse import bacc
from concourse.bass_utils import run_bass_kernel_spmd

B, N, DIM, H, DH = 4, 2048, 1536, 16, 96
HC = H // 2          # heads per core
KT = DIM // 128      # 12 contraction chunks
KO = HC * DH // 128  # 6 output-projection contraction chunks
TT = 512             # moving-dim tile
NTC = N // 128       # 16 token chunks of 128
NQT = N // TT        # 4 query tiles of 512
VALL = HC * DH       # 768 V-projection columns
SCALE = DH ** -0.5
F32 = mybir.dt.float32
F32R = mybir.dt.float32r
BF16 = mybir.dt.bfloat16
IN_NP = ml_dtypes.bfloat16
AF = mybir.ActivationFunctionType
HALF_PI = float(np.pi / 2)
APPROX_RECIP = True


def _build_rope_pt() -> np.ndarray:
    """lhsT for the rotate_half permutation: rot(t) = PT.T @ t.

    Per 32-chunk c (3 chunks), within-chunk index a:
      a <  16: rot[base+a] = -t[base+a+16]
      a >= 16: rot[base+a] = +t[base+a-16]
    """
    P = np.zeros((DH, DH), np.float32)
    for c in range(3):
        base = 32 * c
        for a in range(16):
            P[base + a, base + a + 16] = -1.0
            P[base + 16 + a, base + a] = 1.0
    return np.ascontiguousarray(P.T)


def _emit(ctx, tc, io):
    nc = tc.nc
    xp, wkp, wqp, wvp, woutp, fkT, ropePT, out = io

    # ---- persistent constants -------------------------------------------
    persist = ctx.enter_context(tc.tile_pool(name="persist", bufs=1))
    ropeP_sb = persist.tile([DH, DH], BF16, tag="ropeP")
    nc.sync.dma_start(out=ropeP_sb, in_=ropePT)
    ones1f = persist.tile([1, DH], F32, tag="ones1f")
    nc.vector.memset(ones1f, 1.0)
    ones1 = persist.tile([1, DH], F32R, tag="ones1")
    nc.scalar.copy(out=ones1, in_=ones1f)
    onescol = persist.tile([128, NTC, 1], F32, tag="onescol")
    nc.vector.memset(onescol, 1.0)
    halfpi = persist.tile([DH, 1], F32, tag="halfpi")
    nc.vector.memset(halfpi, HALF_PI)

    # ---- persistent per-head tensors ------------------------------------
    pkq = ctx.enter_context(tc.tile_pool(name="kq", bufs=3))
    k_t = {}
    q_t = {}
    pv = ctx.enter_context(tc.tile_pool(name="v1", bufs=1))
    v1s = [pv.tile([128, NTC, DH + 1], BF16, tag=f"v1_{h}", name=f"v1_{h}")
           for h in range(HC)]
    pho = ctx.enter_context(tc.tile_pool(name="ho", bufs=3))
    ho_t = {}
    phop = ctx.enter_context(tc.tile_pool(name="hop", bufs=1))
    hoP = [phop.tile([128, N], BF16, tag=f"hop{kk}", name=f"hop{kk}")
           for kk in range(KO)]
    pex = ctx.enter_context(tc.tile_pool(name="ex", bufs=4))
    pnrm = ctx.enter_context(tc.tile_pool(name="nrm", bufs=2))
    pnrm1 = ctx.enter_context(tc.tile_pool(name="nrm1", bufs=2))
    pnrm1a = ctx.enter_context(tc.tile_pool(name="nrm1a", bufs=1))

    def norm_start(h, qt, ho_ps, psbc):  # psbc: pool for the bcast tile
        """Evacuate ho PSUM: reciprocal of denom (row DH) + staging copy.
        Returns a finish-thunk to emit later (bc matmul + final scale)."""
        qsl = slice(qt * TT, (qt + 1) * TT)
        rcp96 = pnrm1a.tile([DH + 1, TT], F32, tag="rcp96", name="rcp96")
        rcp_f = pnrm1.tile([1, TT], F32, tag="rcpf", name="rcpf")
        if APPROX_RECIP:
            nc.vector.tensor_copy(out=rcp96[DH:DH + 1, :],
                                  in_=ho_ps[DH:DH + 1, :])
            den_f = pnrm1.tile([1, TT], F32, tag="denf", name="denf")
            nc.sync.dma_start(out=den_f, in_=rcp96[DH:DH + 1, :])
            nc.vector.reciprocal_approx_fast(out=rcp_f, in_=den_f)
        else:
            nc.vector.reciprocal(out=rcp96[DH:DH + 1, :],
                                 in_=ho_ps[DH:DH + 1, :])
            nc.sync.dma_start(out=rcp_f, in_=rcp96[DH:DH + 1, :])
        hoU = pnrm.tile([DH, TT], BF16, tag="hoU", name="hoU")
        nc.vector.tensor_copy(out=hoU, in_=ho_ps[0:DH, :])
        dest = ho_t[h]

        def finish():
            rcp_r = pnrm1.tile([1, TT], F32R, tag="rcpr", name="rcpr")
            nc.scalar.copy(out=rcp_r, in_=rcp_f)
            bc = psbc.tile([DH, TT], F32, tag="sc", name="bc")
            nc.tensor.matmul(out=bc, lhsT=ones1, rhs=rcp_r,
                             start=True, stop=True)
            nc.vector.tensor_mul(out=dest[:, qsl], in0=hoU, in1=bc)
            repack(h, qt)
        return finish

    def repack(h, qt):
        """DMA head h's normalized rows into the 128-row stacked layout."""
        sl = slice(qt * TT, (qt + 1) * TT)
        kk0, off = divmod(DH * h, 128)
        n1 = min(128 - off, DH)
        nc.sync.dma_start(out=hoP[kk0][off:off + n1, sl],
                          in_=ho_t[h][0:n1, sl])
        if n1 < DH:
            nc.sync.dma_start(out=hoP[kk0 + 1][0:DH - n1, sl],
                              in_=ho_t[h][n1:DH, sl])

    def attn_pass(h, pp, psscore, psho, pre=(), psbc=None, inject=(),
                  pre_at=(6, 10)):
        """Attention for head h over query tiles (2pp, 2pp+1).

        pre: finish-thunks from the previous pass's normalization, emitted
        a few key-chunks in so their cross-engine chains never stall the
        PE queue. Returns this pass's finish-thunks."""
        sla = slice((2 * pp) * TT, (2 * pp + 1) * TT)
        slb = slice((2 * pp + 1) * TT, (2 * pp + 2) * TT)
        if pp == 0:
            ho_t[h] = pho.tile([DH, N], BF16, tag="ho", name=f"ho{h}")
        ho_a = psho.tile([DH + 1, TT], F32, tag="hops", name=f"hoa{h}_{pp}")
        ho_b = psho.tile([DH + 1, TT], F32, tag="hops", name=f"hob{h}_{pp}")

        def sc_kc(kc):
            sc = psscore.tile([128, 2 * TT], F32, tag="sc",
                              name=f"sc{h}_{pp}_{kc}")
            lh = k_t[h][:, kc * 128:(kc + 1) * 128]
            nc.tensor.matmul(out=sc[:, 0:TT], lhsT=lh, rhs=q_t[h][:, sla],
                             start=True, stop=True)
            nc.tensor.matmul(out=sc[:, TT:2 * TT], lhsT=lh,
                             rhs=q_t[h][:, slb], start=True, stop=True)
            return sc

        runahead = 2 if psscore.bufs > 2 else 1
        scs = [sc_kc(i) for i in range(runahead)]
        for kc in range(NTC):
            sc = scs.pop(0)
            ex = pex.tile([128, 2 * TT], BF16, tag="ex", name="ex")
            nc.scalar.activation(out=ex, in_=sc, func=AF.Exp, scale=SCALE)
            nxt = kc + len(scs) + 1
            if nxt < NTC:
                scs.append(sc_kc(nxt))
            lv = v1s[h][:, kc, :]
            nc.tensor.matmul(out=ho_a, lhsT=lv, rhs=ex[:, 0:TT],
                             start=(kc == 0), stop=(kc == NTC - 1))
            nc.tensor.matmul(out=ho_b, lhsT=lv, rhs=ex[:, TT:2 * TT],
                             start=(kc == 0), stop=(kc == NTC - 1))
            if kc == pre_at[0] and len(pre) > 0:
                pre[0]()
            if kc == pre_at[1] and len(pre) > 1:
                pre[1]()
            if kc % 4 == 3 and inject:
                inject.pop(0)()
        fa = norm_start(h, 2 * pp, ho_a, psbc or psscore)
        fb = norm_start(h, 2 * pp + 1, ho_b, psbc or psscore)
        return [fa, fb]

    # ---- phase 1: projections with attention interleaved (2-head lag) ---
    with (
        tc.tile_pool(name="px", bufs=1) as px,
        tc.tile_pool(name="wqk", bufs=2) as pwqk,
        tc.tile_pool(name="wv", bufs=1) as pwv,
        tc.tile_pool(name="rope", bufs=2) as prope,
        tc.tile_pool(name="ropeuw", bufs=2) as pruw,
        tc.tile_pool(name="psproj", bufs=2, space="PSUM") as psproj,
    ):
        def load_w(h, eng=None):
            eng = eng or nc.sync
            wk = pwqk.tile([128, KT, DH], BF16, tag="wk", name=f"wk{h}")
            eng.dma_start(out=wk, in_=wkp[:, h])
            wq = pwqk.tile([128, KT, DH], BF16, tag="wq", name=f"wq{h}")
            eng.dma_start(out=wq, in_=wqp[:, h])
            return wk, wq

        # trig tables (queries == keys: one table pair); weight DMAs go
        # out on the scalar queue before the Sin ops so the first
        # projection group is not gated on trig
        fk_sb = px.tile([DH, N], BF16, tag="fk")
        nc.scalar.dma_start(out=fk_sb, in_=fkT)
        w_tiles = {0: load_w(0, nc.scalar)}
        cosk = px.tile([DH, N], BF16, tag="cosk")
        sink = px.tile([DH, N], BF16, tag="sink")
        nc.scalar.activation(out=sink, in_=fk_sb, func=AF.Sin)
        nc.scalar.activation(out=cosk, in_=fk_sb, func=AF.Sin, bias=halfpi)
        xs = []
        for t in range(NQT):
            xt = px.tile([128, KT, TT], BF16, tag=f"xs{t}", name=f"xs{t}")
            if t == 0:
                # per-chunk DMAs: the first projection group's k-th matmul
                # only waits for chunk k, so compute starts ~6us earlier
                for k in range(KT):
                    nc.sync.dma_start(out=xt[:, k, :], in_=xp[:, 0, k, :])
            else:
                nc.sync.dma_start(out=xt, in_=xp[:, t])
            xs.append(xt)
        w_tiles[1] = load_w(1)
        wv_sb = pwv.tile([128, KT, VALL], BF16, tag="wv")
        nc.sync.dma_start(out=wv_sb, in_=wvp)

        def rope(ps, sl, dest):
            """dest = cos*t + sin*(P @ t), t = ps (PSUM [DH, TT])."""
            t_sb = prope.tile([DH, TT], BF16, tag="ropet", name="ropet")
            nc.vector.tensor_copy(out=t_sb, in_=ps)
            rot = psproj.tile([128, TT], F32, tag="pj", name="rot")
            nc.tensor.matmul(out=rot[0:DH, :], lhsT=ropeP_sb, rhs=t_sb,
                             start=True, stop=True)
            u = pruw.tile([DH, TT], F32, tag="ropeu", name="ropeu")
            nc.gpsimd.tensor_mul(out=u, in0=t_sb, in1=cosk[:, sl])
            w = pruw.tile([DH, TT], F32, tag="ropew", name="ropew")
            nc.vector.tensor_mul(out=w, in0=rot[0:DH, :], in1=sink[:, sl])
            nc.gpsimd.tensor_add(out=dest, in0=u, in1=w)

        def proj_jobs(h):
            """Projection of head h as 8 injectable jobs + a flush; the rot
            half of RoPE runs one job late so its PSUM evacuation hides
            under interleaved attention matmuls."""
            wk, wq = w_tiles[h]
            k_t[h] = pkq.tile([DH, N], BF16, tag="kt", name=f"kt{h}")
            q_t[h] = pkq.tile([DH, N], BF16, tag="qt", name=f"qt{h}")
            state = {"pending": None}

            def rot_part(t_sb, sl, dest):
                rot = psproj.tile([128, TT], F32, tag="pj", name="rot")
                nc.tensor.matmul(out=rot[0:DH, :], lhsT=ropeP_sb, rhs=t_sb,
                                 start=True, stop=True)
                u = pruw.tile([DH, TT], F32, tag="ropeu", name="ropeu")
                nc.gpsimd.tensor_mul(out=u, in0=t_sb, in1=cosk[:, sl])
                w = pruw.tile([DH, TT], F32, tag="ropew", name="ropew")
                nc.vector.tensor_mul(out=w, in0=rot[0:DH, :], in1=sink[:, sl])
                nc.gpsimd.tensor_add(out=dest, in0=u, in1=w)

            def mk(w_sb, dest, t):
                def job():
                    sl = slice(t * TT, (t + 1) * TT)
                    ps = psproj.tile([128, TT], F32, tag="pj", name="pj")
                    for k in range(KT):
                        nc.tensor.matmul(
                            out=ps[0:DH, :], lhsT=w_sb[:, k, :],
                            rhs=xs[t][:, k, :],
                            start=(k == 0), stop=(k == KT - 1),
                        )
                    t_sb = prope.tile([DH, TT], BF16, tag="ropet",
                                      name="ropet")
                    nc.vector.tensor_copy(out=t_sb, in_=ps[0:DH, :])
                    if state["pending"] is not None:
                        rot_part(*state["pending"])
                    state["pending"] = (t_sb, sl, dest[:, sl])
                return job

            def flush():
                rot_part(*state["pending"])
                state["pending"] = None

            return ([mk(wk, k_t[h], t) for t in range(NQT)] +
                    [mk(wq, q_t[h], t) for t in range(NQT)], flush)


        # ---- segs 0-1: first two heads + V (own psum pool scope) --------
        with tc.tile_pool(name="vps", bufs=2, space="PSUM") as vpool:
            for h01 in range(2):
                jobs01, flush01 = proj_jobs(h01)
                for j in jobs01:
                    j()
                flush01()
            for tci in range(NTC):
                ps = vpool.tile([128, 2 * TT], F32, tag="v", name="vps")
                for k in range(KT):
                    xk = xs[tci // 4][:, k, (tci % 4) * 128:(tci % 4 + 1) * 128]
                    nc.tensor.matmul(
                        out=ps[:, 0:TT], lhsT=xk, rhs=wv_sb[:, k, 0:TT],
                        start=(k == 0), stop=(k == KT - 1),
                    )
                    nc.tensor.matmul(
                        out=ps[:, TT:VALL], lhsT=xk, rhs=wv_sb[:, k, TT:VALL],
                        start=(k == 0), stop=(k == KT - 1),
                    )
                for h in range(HC):
                    nc.vector.tensor_copy(
                        out=v1s[h][:, tci, 0:DH],
                        in_=ps[:, h * DH:(h + 1) * DH],
                    )
            for h in range(HC):
                nc.scalar.copy(out=v1s[h][:, :, DH:DH + 1], in_=onescol)

        # ---- segs 2-7: remaining heads with attention interleaved -------
        with (
            tc.tile_pool(name="pssc1", bufs=2, space="PSUM") as pssc1,
            tc.tile_pool(name="psho1", bufs=2, space="PSUM") as psho1,
        ):
            sched = {2: [(0, 0), (0, 1)], 3: [(1, 0), (1, 1)],
                     4: [(2, 0), (2, 1)], 5: [(3, 0), (3, 1)],
                     6: [(4, 0), (4, 1), (5, 0)],
                     7: [(5, 1), (6, 0), (6, 1)]}
            fin = []
            for h in range(2, HC):
                w_tiles[h] = load_w(h)
                jobs, flush = proj_jobs(h)
                for ah, pp in sched[h]:
                    fin = attn_pass(ah, pp, pssc1, psho1, pre=fin,
                                    inject=jobs)
                flush()
            for f in fin:
                f()

    # ---- phase 2a: last two heads' attention, 3-deep score buffering ----
    with (
        tc.tile_pool(name="wo", bufs=1) as pwout,
        tc.tile_pool(name="psbcT", bufs=1, space="PSUM") as psbcT,
    ):
        wo_sb = pwout.tile([128, KO, DIM], BF16, tag="wo")
        with (
            tc.tile_pool(name="pssc2", bufs=2, space="PSUM") as pssc2,
            tc.tile_pool(name="psho2", bufs=2, space="PSUM") as psho2,
        ):
            fin = attn_pass(HC - 1, 0, pssc2, psho2, psbc=psbcT,
                            pre_at=(10, 14))
            nc.sync.dma_start(out=wo_sb, in_=woutp)
            fin = attn_pass(HC - 1, 1, pssc2, psho2, pre=fin, psbc=psbcT,
                            pre_at=(10, 14))

        # ---- phase 2b: dense output projection --------------------------
        with (
            tc.tile_pool(name="osb", bufs=3) as posb,
            tc.tile_pool(name="psout", bufs=3, space="PSUM") as psout,
        ):
            for tci in range(NTC):
                if tci in (1, 3) and fin:
                    fin.pop(0)()
                osb = posb.tile([128, DIM], F32, tag="osb", name="osb")
                ps0 = psout.tile([128, TT], F32, tag="ops", name=f"o{tci}_0")
                ps1 = psout.tile([128, TT], F32, tag="ops", name=f"o{tci}_1")
                for kk in range(KO):
                    lh = hoP[kk][:, tci * 128:(tci + 1) * 128]
                    nc.tensor.matmul(out=ps0, lhsT=lh, rhs=wo_sb[:, kk, 0:TT],
                                     start=(kk == 0), stop=(kk == KO - 1))
                    nc.tensor.matmul(out=ps1, lhsT=lh,
                                     rhs=wo_sb[:, kk, TT:2 * TT],
                                     start=(kk == 0), stop=(kk == KO - 1))
                nc.vector.tensor_copy(out=osb[:, 0:TT], in_=ps0)
                nc.sync.dma_start(out=out[tci * 128:(tci + 1) * 128, 0:TT],
                                  in_=osb[:, 0:TT])
                nc.scalar.copy(out=osb[:, TT:2 * TT], in_=ps1)
                nc.sync.dma_start(
                    out=out[tci * 128:(tci + 1) * 128, TT:2 * TT],
                    in_=osb[:, TT:2 * TT])
                ps2 = psout.tile([128, TT], F32, tag="ops", name=f"o{tci}_2")
                for kk in range(KO):
                    nc.tensor.matmul(
                        out=ps2, lhsT=hoP[kk][:, tci * 128:(tci + 1) * 128],
                        rhs=wo_sb[:, kk, 2 * TT:3 * TT],
                        start=(kk == 0), stop=(kk == KO - 1))
                nc.vector.tensor_copy(out=osb[:, 2 * TT:3 * TT], in_=ps2)
                nc.sync.dma_start(
                    out=out[tci * 128:(tci + 1) * 128, 2 * TT:3 * TT],
                    in_=osb[:, 2 * TT:3 * TT])


def build():
    from contextlib import ExitStack

    nc = bacc.Bacc("TRN2", target_bir_lowering=False, debug=False)
    xp = nc.declare_dram_parameter("xp", [128, NQT, KT, TT], BF16, isOutput=False)
    wkp = nc.declare_dram_parameter("wkp", [128, HC, KT, DH], BF16, isOutput=False)
    wqp = nc.declare_dram_parameter("wqp", [128, HC, KT, DH], BF16, isOutput=False)
    wvp = nc.declare_dram_parameter("wvp", [128, KT, VALL], BF16, isOutput=False)
    woutp = nc.declare_dram_parameter("woutp", [128, KO, DIM], BF16, isOutput=False)
    fkT = nc.declare_dram_parameter("fkT", [DH, N], BF16, isOutput=False)
    ropePT = nc.declare_dram_parameter("ropePT", [DH, DH], BF16, isOutput=False)
    out = nc.declare_dram_parameter("out", [N, DIM], F32, isOutput=True)
    io = tuple(t[:] for t in (xp, wkp, wqp, wvp, woutp, fkT, ropePT, out))
    with ExitStack() as ctx:
        tc = ctx.enter_context(tile.TileContext(nc))
        _emit(ctx, tc, io)
    nc.finalize()
    return nc


def make_in_maps(x, f1, f2, f3, Wqkv, Wout, bout):
    x = np.asarray(x, np.float32)
    fcat = np.concatenate(
        [np.asarray(f1, np.float32), np.asarray(f2, np.float32),
         np.asarray(f3, np.float32)], axis=1,
    )  # [N, DH]
    fkT_np = np.ascontiguousarray(fcat.T).astype(IN_NP)
    PT = _build_rope_pt().astype(IN_NP)
    Wqkv = np.asarray(Wqkv, np.float32)
    Wout = np.asarray(Wout, np.float32)
    HDH = H * DH
    xps = []
    for b in range(B):
        xT = np.ascontiguousarray(x[b].T)  # [DIM, N]
        xps.append(np.ascontiguousarray(
            xT.reshape(KT, 128, NQT, TT).transpose(1, 2, 0, 3)).astype(IN_NP))
    halves = []
    for hh in range(2):
        hs = hh * HC
        wq = Wqkv[:, hs * DH:(hs + HC) * DH]
        wk = Wqkv[:, HDH + hs * DH:HDH + (hs + HC) * DH]
        wv = Wqkv[:, 2 * HDH + hs * DH:2 * HDH + (hs + HC) * DH]
        halves.append(dict(
            wqp=np.ascontiguousarray(
                wq.reshape(KT, 128, HC, DH).transpose(1, 2, 0, 3)).astype(IN_NP),
            wkp=np.ascontiguousarray(
                wk.reshape(KT, 128, HC, DH).transpose(1, 2, 0, 3)).astype(IN_NP),
            wvp=np.ascontiguousarray(
                wv.reshape(KT, 128, HC * DH).transpose(1, 0, 2)).astype(IN_NP),
            woutp=np.ascontiguousarray(
                Wout[hs * DH:(hs + HC) * DH, :].reshape(KO, 128, DIM)
                .transpose(1, 0, 2)).astype(IN_NP),
        ))
    in_maps = []
    for c in range(8):
        b, hh = divmod(c, 2)
        in_maps.append(dict(
            xp=xps[b], fkT=fkT_np, ropePT=PT, **halves[hh],
        ))
    return in_maps


_NC_CACHE = None


def kernel(x, f1, f2, f3, Wqkv, Wout, bout, _trace=False):
    global _NC_CACHE
    if _NC_CACHE is None:
        _NC_CACHE = build()
    nc = _NC_CACHE
    in_maps = make_in_maps(x, f1, f2, f3, Wqkv, Wout, bout)
    res = run_bass_kernel_spmd(nc, in_maps, list(range(8)), trace=_trace)
    out = np.empty((B, N, DIM), np.float32)
    for b in range(B):
        out[b] = res.results[2 * b]["out"] + res.results[2 * b + 1]["out"]
    out += np.asarray(bout, np.float32)
    if _trace:
        return out, res
    return out
